# revision 5
# baseline (speedup 1.0000x reference)
"""CrossAttention kernel for 8 TRN2 NeuronCores (Bass/Tile).

Reference computation (per batch b):
    q = x @ Wq ; k = ctx @ Wk ; v = ctx @ Wv        (heads H=8, dh=64)
    attn = softmax(q k^T / sqrt(dh)) ; o = attn @ v
    out = o @ Wo + bo

Sharding (8 cores): core c -> (batch b = c//2, head-group hg = c%2).
Each core handles 4 heads of one batch over the full sequence; the two
head-group partial outputs per batch are summed on the host (Wo is
sliced by rows, so partials add exactly).

v2 schedule (from baseline trace analysis, 185us):
  - ramp: the critical first tensors (xq0/cq0 by kt-slice, wq/wk by
    half) are spread over the 3 DMA-capable queues so the first
    projection matmul starts as soon as its first slices land.
  - exp tiles [128,1024]: ACT (table exp) for m0/m1/m15 and ~9/16,
    DVE Schraudolph for the rest.  m0/m1 on ACT lets DVE run the
    previous block's normalize in parallel at block boundaries.
  - AV lags the S stream by TWO m-tiles so AV never stalls on exp.
  - normalize: reciprocal straight off PSUM (no staging copy), order
    recip0/TT0/recip1/TT1 so po[0] frees earliest; Y and Q-projection
    prefetches are emitted at block boundaries as PE filler while DVE
    normalizes (GPSIMD cannot touch PSUM on TRN2, so all psum-sourced
    elementwise work lives on ACT/DVE only).
  - Y: four [128,512] psum chains per chunk (kt0 reads the pr=0 half
    of Ocat, ready a block early), copies alternate ACT/DVE, 4 output
    DMAs per chunk on the SP queue.
  - psum: 2x [128,1024] S-pair tiles, 2x [128,512] po, 2x [128,512]
    projection ring = 8 banks exactly (projection ring doubled vs
    baseline to kill the block-0/1 WAR stalls).
"""

import os

import numpy as np

import concourse.bass as bass
import concourse.mybir as mybir
import concourse.tile as tile
from concourse import bacc
from concourse.bass_utils import run_bass_kernel_spmd

F16 = mybir.dt.float16
F32 = mybir.dt.float32
I16 = mybir.dt.int16

D = 512          # model dim
N = 2048         # query seq len
M = 2048         # key seq len
HPC = 4          # heads per core
DH = 64          # head dim
DS = HPC * DH    # per-core inner dim = 256
SCALE = 1.0 / 8.0  # 1/sqrt(64)
P = 128
KT_D = D // P    # 4 k-tiles over model dim
NS = 512         # query sub-chunk / seq quarter
LOG2E = 1.4426950408889634
SCH_SCALE = 1024.0 * LOG2E * SCALE
SCH_BIAS = 15315.5

_NP16 = np.float16

# exp engine per m-tile: True = ACT table exp, False = DVE Schraudolph.
# m0/m1 on ACT (DVE normalizes the previous block then), m15 on ACT
# (tail latency).
_EXP_ACT = {0, 1, 4, 6, 8, 10, 12, 14, 15}


def _build_nc():
    nc = bacc.Bacc(None, target_bir_lowering=False)

    xq = [nc.declare_dram_parameter(f"xq{i}", [P, KT_D, NS], F16,
                                    isOutput=False) for i in range(4)]
    cq = [nc.declare_dram_parameter(f"cq{i}", [P, KT_D, NS], F16,
                                    isOutput=False) for i in range(4)]
    wq = nc.declare_dram_parameter("wq", [P, KT_D, DS], F16, isOutput=False)
    wk = nc.declare_dram_parameter("wk", [P, KT_D, DS], F16, isOutput=False)
    wv = nc.declare_dram_parameter("wv", [P, KT_D, DS], F16, isOutput=False)
    wo = nc.declare_dram_parameter("wo", [P, DS // P, D], F16, isOutput=False)
    yT = nc.declare_dram_parameter("yT", [N // NS, P, D // P, NS], F16,
                                   isOutput=True)

    with tile.TileContext(nc) as tc:
        _emit(tc, xq, cq, wq, wk, wv, wo, yT)
    nc.finalize()
    return nc


def _emit(tc, xq, cq, wq, wk, wv, wo, yT):
    nc = tc.nc
    MT = M // P          # 16 m-tiles over keys
    NCH = 2 * NS         # psum tile width for a head-pair
    EXP = mybir.ActivationFunctionType.Exp

    from contextlib import ExitStack

    with ExitStack() as ctx:
        const = ctx.enter_context(tc.tile_pool(name="const", bufs=1))
        work = ctx.enter_context(tc.tile_pool(name="work", bufs=6))
        rcp = ctx.enter_context(tc.tile_pool(name="rcp", bufs=4))
        yout = ctx.enter_context(tc.tile_pool(name="yout", bufs=4))
        ps_s = ctx.enter_context(tc.tile_pool(name="ps_s", bufs=2, space="PSUM"))
        ps_o = ctx.enter_context(tc.tile_pool(name="ps_o", bufs=2, space="PSUM"))
        ps_p = ctx.enter_context(tc.tile_pool(name="ps_p", bufs=2, space="PSUM"))

        # ---- resident SBUF tensors ----
        xT_q = [const.tile([P, KT_D, NS], F16, name=f"xT{i}") for i in range(4)]
        cT_q = [const.tile([P, KT_D, NS], F16, name=f"cT{i}") for i in range(4)]
        wq_sb = const.tile([P, KT_D, DS], F16)
        wk_sb = const.tile([P, KT_D, DS], F16)
        wv_sb = const.tile([P, KT_D, DS], F16)
        wo_sb = const.tile([P, DS // P, D], F16)
        QT_sb = const.tile([P, DS // P, N], F16)
        KT_sb = const.tile([P, DS // P, M], F16)
        # per (m-tile, head): 128 stationary columns = [V_h (64) | ones (64)]
        Vp_sb = const.tile([P, MT, HPC, P], F16)
        Ocat = const.tile([P, DS // P, N], F16)

        # ---- input DMAs: kt-sliced criticals over the 3 DMA-capable
        # queues so the first projection matmuls start as soon as their
        # first slices land. (Only SP/Activation/gpsimd can issue DMAs.)
        for kt in range(KT_D):
            nc.sync.dma_start(xT_q[0][:, kt, :], xq[0][:, kt, :])
            nc.gpsimd.dma_start(cT_q[0][:, kt, :], cq[0][:, kt, :])
        nc.scalar.dma_start(wq_sb[:, 0:2, :], wq[:, 0:2, :])
        nc.scalar.dma_start(wk_sb[:, 0:2, :], wk[:, 0:2, :])
        nc.scalar.dma_start(wq_sb[:, 2:4, :], wq[:, 2:4, :])
        nc.scalar.dma_start(wk_sb[:, 2:4, :], wk[:, 2:4, :])
        nc.scalar.dma_start(wv_sb[:], wv[:])
        nc.sync.dma_start(cT_q[1][:], cq[1][:])
        nc.gpsimd.dma_start(xT_q[1][:], xq[1][:])
        nc.vector.memset(Vp_sb[:, :, :, DH:P], 1.0)
        deferred_dma = {"done": False}

        def emit_deferred_dma():
            if deferred_dma["done"]:
                return
            deferred_dma["done"] = True
            nc.sync.dma_start(cT_q[2][:], cq[2][:])
            nc.sync.dma_start(cT_q[3][:], cq[3][:])
            nc.sync.dma_start(xT_q[2][:], xq[2][:])
            nc.sync.dma_start(xT_q[3][:], xq[3][:])
            nc.sync.dma_start(wo_sb[:], wo[:])

        # ---- projections, prefetched ahead of use ----
        proj_done = set()

        def emit_q(pr, ch):
            if ("q", pr, ch) in proj_done:
                return
            proj_done.add(("q", pr, ch))
            ps = ps_p.tile([P, NS], F32, tag="psp", name=f"q{pr}{ch}")
            for kt in range(KT_D):
                nc.tensor.matmul(
                    ps[:, :NS],
                    lhsT=wq_sb[:, kt, pr * P:(pr + 1) * P],
                    rhs=xT_q[ch][:, kt, :],
                    start=(kt == 0),
                    stop=(kt == KT_D - 1),
                )
            nc.scalar.copy(
                QT_sb[:, pr, ch * NS:(ch + 1) * NS], ps[:, :NS]
            )

        def emit_k(pr, dt):
            if ("k", pr, dt) in proj_done:
                return
            proj_done.add(("k", pr, dt))
            ps = ps_p.tile([P, NS], F32, tag="psp", name=f"k{pr}{dt}")
            for kt in range(KT_D):
                nc.tensor.matmul(
                    ps[:, :NS],
                    lhsT=wk_sb[:, kt, pr * P:(pr + 1) * P],
                    rhs=cT_q[dt][:, kt, :],
                    start=(kt == 0),
                    stop=(kt == KT_D - 1),
                )
            nc.vector.tensor_copy(
                KT_sb[:, pr, dt * NS:(dt + 1) * NS], ps[:, :NS]
            )
            emit_deferred_dma()

        def emit_v(mp):
            # projects a PAIR of m-tiles (2*mp, 2*mp+1) in one psum tile
            if ("v", mp) in proj_done:
                return
            proj_done.add(("v", mp))
            ps = ps_p.tile([P, NS], F32, tag="psp", name=f"v{mp}")
            for mi in range(2):
                mt = 2 * mp + mi
                for kt in range(KT_D):
                    nc.tensor.matmul(
                        ps[:, mi * DS:(mi + 1) * DS],
                        lhsT=cT_q[mt // 4][:, kt, (mt % 4) * P:(mt % 4 + 1) * P],
                        rhs=wv_sb[:, kt, :],
                        start=(kt == 0),
                        stop=(kt == KT_D - 1),
                    )
            eng = nc.vector if mp % 2 == 0 else nc.scalar
            if eng is nc.scalar:
                eng.copy(
                    Vp_sb[:, 2 * mp:2 * mp + 2, :, 0:DH],
                    ps[:, 0:2 * DS].rearrange("p (m h d) -> p m h d", h=HPC, m=2),
                )
            else:
                eng.tensor_copy(
                    Vp_sb[:, 2 * mp:2 * mp + 2, :, 0:DH],
                    ps[:, 0:2 * DS].rearrange("p (m h d) -> p m h d", h=HPC, m=2),
                )

        def emit_y(nch):
            for dt4 in range(D // P):
                ps = ps_p.tile([P, NS], F32, tag="psp", name=f"y{nch}{dt4}")
                for kt in range(DS // P):
                    nc.tensor.matmul(
                        ps[:, :NS],
                        lhsT=wo_sb[:, kt, dt4 * P:(dt4 + 1) * P],
                        rhs=Ocat[:, kt, nch * NS:(nch + 1) * NS],
                        start=(kt == 0),
                        stop=(kt == DS // P - 1),
                    )
                yt = yout.tile([P, NS], F16, tag="y")
                if dt4 % 2 == 0:
                    nc.scalar.copy(yt[:], ps[:, :NS])
                else:
                    nc.vector.tensor_copy(yt[:], ps[:, :NS])
                nc.sync.dma_start(yT[nch][:, dt4, :], yt[:])

        # ---- software-pipelined attention ----
        blocks = [(nch, pr) for nch in range(N // NS) for pr in range(HPC // 2)]
        NB = len(blocks)

        st_tiles = {}
        e_tiles = {}
        av_done = set()
        po_cur = {}

        def emit_exp(bi, mt):
            st = st_tiles.pop((bi, mt))
            e = work.tile([P, NCH], F16, tag="e")
            if mt in _EXP_ACT or (bi + 1 == NB and mt >= 13):
                nc.scalar.activation(e[:], st[:], EXP, scale=SCALE)
            else:
                nc.vector.tensor_scalar(
                    e.bitcast(I16)[:], st[:], SCH_SCALE, SCH_BIAS,
                    mybir.AluOpType.mult, mybir.AluOpType.add,
                )
            e_tiles[(bi, mt)] = e

        def emit_AV(bi, mt):
            if (bi, mt) in av_done:
                return
            av_done.add((bi, mt))
            nch, pr = blocks[bi]
            po = po_cur[bi]
            e = e_tiles.pop((bi, mt))
            for i in range(2):
                nc.tensor.matmul(
                    po[i][:],
                    lhsT=Vp_sb[:, mt, 2 * pr + i, :],
                    rhs=e[:, i * NS:(i + 1) * NS],
                    start=(mt == 0),
                    stop=(mt == MT - 1),
                )

        def emit_S(bi, mt):
            nch, pr = blocks[bi]
            n0 = nch * NS
            emit_q(pr, nch)
            emit_k(pr, mt // 4)
            st = ps_s.tile([P, NCH], F32, tag="ps", name=f"s{bi}_{mt}")
            for i in range(2):
                dp = i * DH
                nc.tensor.matmul(
                    st[:, i * NS:(i + 1) * NS],
                    lhsT=KT_sb[dp:dp + DH, pr, mt * P:(mt + 1) * P],
                    rhs=QT_sb[dp:dp + DH, pr, n0:n0 + NS],
                    start=True,
                    stop=True,
                )
            st_tiles[(bi, mt)] = st

        def emit_normalize(bi):
            # stage sums to SBUF (custom-DVE recip can't read PSUM on
            # HW), then recip + multiply; order chain0 fully first so
            # po[0] frees earliest for the next block's AV(m0).
            nch, pr = blocks[bi]
            n0 = nch * NS
            po = po_cur.pop(bi)
            for i in range(2):
                sc = rcp.tile([DH, NS], F32, tag="sc")
                if i == 0:
                    nc.scalar.copy(sc[:], po[i][DH:P, :])
                else:
                    nc.vector.tensor_copy(sc[:], po[i][DH:P, :])
                rc = rcp.tile([DH, NS], F32, tag="rc")
                nc.vector.reciprocal_approx_fast(rc[:], sc[:])
                nc.vector.tensor_tensor(
                    Ocat[i * DH:(i + 1) * DH, pr, n0:n0 + NS],
                    po[i][0:DH, :],
                    rc[:],
                    mybir.AluOpType.mult,
                )

        # prologue
        emit_S(0, 0)

        for bi, (nch, pr) in enumerate(blocks):
            po = [
                ps_o.tile([P, NS], F32, tag="po", name=f"po{bi}_{i}")
                for i in range(2)
            ]
            po_cur[bi] = po
            lag = 1 if bi + 1 == NB else 2
            for mt in range(MT):
                emit_exp(bi, mt)

                # PE: next S first (never behind a DMA-gated projection
                # or the exp-gated AV), then boundary filler work that
                # covers the previous block's normalize, then the
                # lagging AV.
                if mt + 1 < MT:
                    emit_S(bi, mt + 1)
                elif bi + 1 < NB:
                    emit_S(bi + 1, 0)
                if bi == 0:
                    if mt % 2 == 0:
                        emit_v(mt // 2)
                    if mt in (3, 5, 7, 9):        # K for head-pair 1
                        emit_k(1, (mt - 3) // 2)
                    elif mt == 11:
                        emit_q(1, 0)
                else:
                    if mt == 0 and bi + 2 < NB:   # Q for the block after next
                        emit_q(blocks[bi + 2][1], blocks[bi + 2][0])
                    if mt == 0 and bi % 2 == 1 and bi >= 3:
                        emit_y(bi // 2 - 1)       # Y: chunk finished 2 blocks ago
                if mt >= lag:
                    emit_AV(bi, mt - lag)

            emit_AV(bi, MT - 2)
            emit_AV(bi, MT - 1)
            emit_normalize(bi)

            if pr == 1 and nch == N // NS - 1:
                emit_y(nch)                       # last chunk: tail


def _install_ntff_hook():
    """Best-effort NTFF profiling under axon: provide the antenv.axon_hooks
    shim the boot code looks for, and avoid the artifact upload."""
    try:
        import sys
        import types

        import concourse.bass_utils as bu

        bu.upload_artifacts = lambda d: d  # no S3 in this sandbox
        try:
            from antenv.axon_hooks import get_axon_ntff_profile_hook  # noqa: F401
            return  # already present
        except ImportError:
            pass
        import antenv
        from trn_agent_boot.trn_boot import _ntff_profile_via_ctypes

        mod = types.ModuleType("antenv.axon_hooks")
        _state = {"hook": _ntff_profile_via_ctypes("/opt/axon/libaxon_pjrt.so")}
        mod.set_axon_ntff_profile_hook = lambda h: _state.__setitem__("hook", h)
        mod.get_axon_ntff_profile_hook = lambda: _state["hook"]
        sys.modules["antenv.axon_hooks"] = mod
        antenv.axon_hooks = mod
    except Exception as e:  # pragma: no cover
        print(f"ntff hook install failed ({e}); running without trace")


def _swizzle_dn(a):
    """[D-like, n] -> [128, D/128, n] contiguous (partition-major)."""
    d, n = a.shape
    return np.ascontiguousarray(
        a.reshape(d // P, P, n).transpose(1, 0, 2)).astype(_NP16)


def kernel(x, context, Wq, Wk, Wv, Wo, bo):
    x = np.asarray(x, dtype=np.float32)
    context = np.asarray(context, dtype=np.float32)
    Wq = np.asarray(Wq, dtype=np.float32)
    Wk = np.asarray(Wk, dtype=np.float32)
    Wv = np.asarray(Wv, dtype=np.float32)
    Wo = np.asarray(Wo, dtype=np.float32)
    bo = np.asarray(bo, dtype=np.float32)
    B = x.shape[0]

    in_maps = []
    for c in range(8):
        b, hg = c // 2, c % 2
        sl = slice(hg * DS, (hg + 1) * DS)
        xT = _swizzle_dn(x[b].T)        # [128, 4, 2048]
        cT = _swizzle_dn(context[b].T)
        m = {
            "wq": _swizzle_dn(Wq[:, sl]),
            "wk": _swizzle_dn(Wk[:, sl]),
            "wv": _swizzle_dn(Wv[:, sl]),
            "wo": _swizzle_dn(Wo[sl, :]),
        }
        for i in range(4):
            m[f"xq{i}"] = np.ascontiguousarray(xT[:, :, i * NS:(i + 1) * NS])
            m[f"cq{i}"] = np.ascontiguousarray(cT[:, :, i * NS:(i + 1) * NS])
        in_maps.append(m)

    nc = _build_nc()
    trace = bool(int(os.environ.get("BASS_KERNEL_TRACE", "0")))
    if trace:
        _install_ntff_hook()
    res = run_bass_kernel_spmd(nc, in_maps, list(range(8)), trace=trace)
    if trace and res.exec_time_ns is not None:
        print(f"HW exec time: {res.exec_time_ns} ns")

    out = np.empty((B, N, D), dtype=np.float32)
    for b in range(B):
        # yT: [nch, p, dt, ns] -> y[d, n] with d = dt*128+p, n = nch*512+ns
        yt = (res.results[2 * b]["yT"].astype(np.float32)
              + res.results[2 * b + 1]["yT"].astype(np.float32))
        y = yt.transpose(2, 1, 0, 3).reshape(D, N)
        out[b] = y.T + bo[None, :]
    return out


# revision 7
# speedup vs baseline: 1.0202x; 1.0202x over previous
"""CrossAttention kernel for 8 TRN2 NeuronCores (Bass/Tile).

Reference computation (per batch b):
    q = x @ Wq ; k = ctx @ Wk ; v = ctx @ Wv        (heads H=8, dh=64)
    attn = softmax(q k^T / sqrt(dh)) ; o = attn @ v
    out = o @ Wo + bo

Sharding (8 cores): core c -> (batch b = c//2, head-group hg = c%2).
Each core handles 4 heads of one batch over the full sequence; the two
head-group partial outputs per batch are summed on the host (Wo is
sliced by rows, so partials add exactly).

v3 schedule (from v2 trace analysis):
  - the whole attention inner loop runs in PE ROW GROUPS: the S pair
    (64-row stationaries at partition 0/64) and the AV matmuls split
    into two 64-key halves per head, ordered so each row group's
    stream alternates psum banks.  Full-array<->row-group transitions
    cost ~100ns each way (LDWEIGHTS cannot be pulled ahead when row
    groups conflict); an all-row-group stream avoids ~200ns per m-tile.
  - po is a ring of FOUR [128,512] psum banks, so block b+1's AV
    accumulators are different banks from block b's: the previous
    block's normalize moves to the MIDDLE of the next block (staged
    sums -> one [128,512] reciprocal -> two tensor_tensor multiplies,
    spread over mt=6/8/10) and the block boundary carries no
    normalize work at all.  Projections share the S-pair psum ring.
  - exp tiles [128,1024]: ACT table exp for {0,1,4,6,8,10,12,14,15},
    DVE Schraudolph for the rest; AV lags S by two m-tiles.
  - ramp: kt-sliced xq0/cq0 + halved wq/wk over the 3 DMA queues; the
    ones-memset is split so it blocks neither the DVE's first copies
    nor gpsimd's DMA issues.
  - Y: four [128,512] chains per chunk at mt==5, copies ACT/DVE
    alternating, DMAs split across the SP and gpsimd queues.
"""

import os

import numpy as np

import concourse.bass as bass
import concourse.mybir as mybir
import concourse.tile as tile
from concourse import bacc
from concourse.bass_utils import run_bass_kernel_spmd

F16 = mybir.dt.float16
F32 = mybir.dt.float32
I16 = mybir.dt.int16

D = 512          # model dim
N = 2048         # query seq len
M = 2048         # key seq len
HPC = 4          # heads per core
DH = 64          # head dim
DS = HPC * DH    # per-core inner dim = 256
SCALE = 1.0 / 8.0  # 1/sqrt(64)
P = 128
KT_D = D // P    # 4 k-tiles over model dim
NS = 512         # query sub-chunk / seq quarter
LOG2E = 1.4426950408889634
SCH_SCALE = 1024.0 * LOG2E * SCALE
SCH_BIAS = 15315.5

_NP16 = np.float16

# exp engine per m-tile: True = ACT table exp, False = DVE Schraudolph.
_EXP_ACT = {0, 1, 4, 6, 8, 10, 12, 14, 15}

AV_SPLIT = False  # AV as 2x 64-key row-group halves per head


def _build_nc():
    nc = bacc.Bacc(None, target_bir_lowering=False)

    xq = [nc.declare_dram_parameter(f"xq{i}", [P, KT_D, NS], F16,
                                    isOutput=False) for i in range(4)]
    cq = [nc.declare_dram_parameter(f"cq{i}", [P, KT_D, NS], F16,
                                    isOutput=False) for i in range(4)]
    wq = nc.declare_dram_parameter("wq", [P, KT_D, DS], F16, isOutput=False)
    wk = nc.declare_dram_parameter("wk", [P, KT_D, DS], F16, isOutput=False)
    wv = nc.declare_dram_parameter("wv", [P, KT_D, DS], F16, isOutput=False)
    wo = nc.declare_dram_parameter("wo", [P, DS // P, D], F16, isOutput=False)
    yT = nc.declare_dram_parameter("yT", [N // NS, P, D // P, NS], F16,
                                   isOutput=True)

    with tile.TileContext(nc) as tc:
        _emit(tc, xq, cq, wq, wk, wv, wo, yT)
    nc.finalize()
    return nc


def _emit(tc, xq, cq, wq, wk, wv, wo, yT):
    nc = tc.nc
    MT = M // P          # 16 m-tiles over keys
    NCH = 2 * NS
    EXP = mybir.ActivationFunctionType.Exp

    from contextlib import ExitStack

    with ExitStack() as ctx:
        const = ctx.enter_context(tc.tile_pool(name="const", bufs=1))
        work = ctx.enter_context(tc.tile_pool(name="work", bufs=6))
        rcp = ctx.enter_context(tc.tile_pool(name="rcp", bufs=4))
        yout = ctx.enter_context(tc.tile_pool(name="yout", bufs=4))
        ps_s = ctx.enter_context(tc.tile_pool(name="ps_s", bufs=2, space="PSUM"))
        ps_o = ctx.enter_context(tc.tile_pool(name="ps_o", bufs=4, space="PSUM"))

        # ---- resident SBUF tensors ----
        xT_q = [const.tile([P, KT_D, NS], F16, name=f"xT{i}") for i in range(4)]
        cT_q = [const.tile([P, KT_D, NS], F16, name=f"cT{i}") for i in range(4)]
        wq_sb = const.tile([P, KT_D, DS], F16)
        wk_sb = const.tile([P, KT_D, DS], F16)
        wv_sb = const.tile([P, KT_D, DS], F16)
        wo_sb = const.tile([P, DS // P, D], F16)
        QT_sb = const.tile([P, DS // P, N], F16)
        KT_sb = const.tile([P, DS // P, M], F16)
        # per (m-tile, head): 128 stationary columns = [V_h (64) | ones (64)]
        Vp_sb = const.tile([P, MT, HPC, P], F16)
        Ocat = const.tile([P, DS // P, N], F16)

        # ---- input DMAs: kt-sliced criticals over the 3 DMA-capable
        # queues so the first projection matmuls start as soon as their
        # first slices land. (Only SP/Activation/gpsimd can issue DMAs.)
        nc.vector.memset(Vp_sb[:, 6:MT, :, DH:P], 1.0)
        for kt in range(KT_D):
            nc.sync.dma_start(xT_q[0][:, kt, :], xq[0][:, kt, :])
            nc.gpsimd.dma_start(cT_q[0][:, kt, :], cq[0][:, kt, :])
        nc.scalar.dma_start(wq_sb[:, 0:2, :], wq[:, 0:2, :])
        nc.scalar.dma_start(wk_sb[:, 0:2, :], wk[:, 0:2, :])
        nc.scalar.dma_start(wq_sb[:, 2:4, :], wq[:, 2:4, :])
        nc.scalar.dma_start(wk_sb[:, 2:4, :], wk[:, 2:4, :])
        nc.scalar.dma_start(wv_sb[:], wv[:])
        nc.sync.dma_start(cT_q[1][:], cq[1][:])
        nc.gpsimd.dma_start(xT_q[1][:], xq[1][:])
        nc.gpsimd.memset(Vp_sb[:, 0:6, :, DH:P], 1.0)
        deferred_dma = {"done": False}

        def emit_deferred_dma():
            if deferred_dma["done"]:
                return
            deferred_dma["done"] = True
            nc.sync.dma_start(cT_q[2][:], cq[2][:])
            nc.sync.dma_start(cT_q[3][:], cq[3][:])
            nc.sync.dma_start(xT_q[2][:], xq[2][:])
            nc.sync.dma_start(xT_q[3][:], xq[3][:])
            nc.sync.dma_start(wo_sb[:], wo[:])

        # ---- projections: psum from the shared ps_s ring ----
        proj_done = set()

        def emit_q(pr, ch):
            if ("q", pr, ch) in proj_done:
                return
            proj_done.add(("q", pr, ch))
            ps = ps_s.tile([P, NCH], F32, tag="ps", name=f"q{pr}{ch}")
            for kt in range(KT_D):
                nc.tensor.matmul(
                    ps[:, :NS],
                    lhsT=wq_sb[:, kt, pr * P:(pr + 1) * P],
                    rhs=xT_q[ch][:, kt, :],
                    start=(kt == 0),
                    stop=(kt == KT_D - 1),
                )
            nc.scalar.copy(
                QT_sb[:, pr, ch * NS:(ch + 1) * NS], ps[:, :NS]
            )

        def emit_k(pr, dt):
            if ("k", pr, dt) in proj_done:
                return
            proj_done.add(("k", pr, dt))
            ps = ps_s.tile([P, NCH], F32, tag="ps", name=f"k{pr}{dt}")
            for kt in range(KT_D):
                nc.tensor.matmul(
                    ps[:, :NS],
                    lhsT=wk_sb[:, kt, pr * P:(pr + 1) * P],
                    rhs=cT_q[dt][:, kt, :],
                    start=(kt == 0),
                    stop=(kt == KT_D - 1),
                )
            nc.vector.tensor_copy(
                KT_sb[:, pr, dt * NS:(dt + 1) * NS], ps[:, :NS]
            )
            emit_deferred_dma()

        def emit_v(mp):
            # projects a PAIR of m-tiles (2*mp, 2*mp+1) in one psum tile
            if ("v", mp) in proj_done:
                return
            proj_done.add(("v", mp))
            ps = ps_s.tile([P, NCH], F32, tag="ps", name=f"v{mp}")
            for mi in range(2):
                mt = 2 * mp + mi
                for kt in range(KT_D):
                    nc.tensor.matmul(
                        ps[:, mi * DS:(mi + 1) * DS],
                        lhsT=cT_q[mt // 4][:, kt, (mt % 4) * P:(mt % 4 + 1) * P],
                        rhs=wv_sb[:, kt, :],
                        start=(kt == 0),
                        stop=(kt == KT_D - 1),
                    )
            eng = nc.vector if mp % 2 == 0 else nc.scalar
            if eng is nc.scalar:
                eng.copy(
                    Vp_sb[:, 2 * mp:2 * mp + 2, :, 0:DH],
                    ps[:, 0:2 * DS].rearrange("p (m h d) -> p m h d", h=HPC, m=2),
                )
            else:
                eng.tensor_copy(
                    Vp_sb[:, 2 * mp:2 * mp + 2, :, 0:DH],
                    ps[:, 0:2 * DS].rearrange("p (m h d) -> p m h d", h=HPC, m=2),
                )

        def emit_y(nch):
            for dt4 in range(D // P):
                ps = ps_s.tile([P, NCH], F32, tag="ps", name=f"y{nch}{dt4}")
                for kt in range(DS // P):
                    nc.tensor.matmul(
                        ps[:, :NS],
                        lhsT=wo_sb[:, kt, dt4 * P:(dt4 + 1) * P],
                        rhs=Ocat[:, kt, nch * NS:(nch + 1) * NS],
                        start=(kt == 0),
                        stop=(kt == DS // P - 1),
                    )
                yt = yout.tile([P, NS], F16, tag="y")
                if dt4 % 2 == 0:
                    nc.scalar.copy(yt[:], ps[:, :NS])
                else:
                    nc.vector.tensor_copy(yt[:], ps[:, :NS])
                eng = nc.sync if dt4 < 2 else nc.gpsimd
                eng.dma_start(yT[nch][:, dt4, :], yt[:])

        # ---- software-pipelined attention ----
        blocks = [(nch, pr) for nch in range(N // NS) for pr in range(HPC // 2)]
        NB = len(blocks)

        st_tiles = {}
        e_tiles = {}
        av_done = set()
        po_cur = {}
        norm_state = {}

        def emit_exp(bi, mt):
            st = st_tiles.pop((bi, mt))
            e = work.tile([P, NCH], F16, tag="e")
            if mt in _EXP_ACT or (bi + 1 == NB and mt >= 13):
                nc.scalar.activation(e[:], st[:], EXP, scale=SCALE)
            else:
                nc.vector.tensor_scalar(
                    e.bitcast(I16)[:], st[:], SCH_SCALE, SCH_BIAS,
                    mybir.AluOpType.mult, mybir.AluOpType.add,
                )
            e_tiles[(bi, mt)] = e

        def emit_AV(bi, mt):
            if (bi, mt) in av_done:
                return
            av_done.add((bi, mt))
            nch, pr = blocks[bi]
            po = po_cur[bi]
            e = e_tiles.pop((bi, mt))
            st, sp = (mt == 0), (mt == MT - 1)
            V0 = Vp_sb[:, mt, 2 * pr, :]
            V1 = Vp_sb[:, mt, 2 * pr + 1, :]
            if AV_SPLIT:
                # two 64-key row-group halves per head; each row group's
                # stream alternates po banks so concurrent halves never
                # write the same bank.
                nc.tensor.matmul(po[0][:], lhsT=V0[0:DH, :],
                                 rhs=e[0:DH, 0:NS], start=st, stop=False)
                nc.tensor.matmul(po[1][:], lhsT=V1[DH:P, :],
                                 rhs=e[DH:P, NS:NCH], start=st, stop=False)
                nc.tensor.matmul(po[1][:], lhsT=V1[0:DH, :],
                                 rhs=e[0:DH, NS:NCH], start=False, stop=sp)
                nc.tensor.matmul(po[0][:], lhsT=V0[DH:P, :],
                                 rhs=e[DH:P, 0:NS], start=False, stop=sp)
            else:
                nc.tensor.matmul(po[0][:], lhsT=V0[:], rhs=e[:, 0:NS],
                                 start=st, stop=sp)
                nc.tensor.matmul(po[1][:], lhsT=V1[:], rhs=e[:, NS:NCH],
                                 start=st, stop=sp)

        def emit_S(bi, mt):
            nch, pr = blocks[bi]
            n0 = nch * NS
            emit_q(pr, nch)
            emit_k(pr, mt // 4)
            st = ps_s.tile([P, NCH], F32, tag="ps", name=f"s{bi}_{mt}")
            for i in range(2):
                dp = i * DH
                nc.tensor.matmul(
                    st[:, i * NS:(i + 1) * NS],
                    lhsT=KT_sb[dp:dp + DH, pr, mt * P:(mt + 1) * P],
                    rhs=QT_sb[dp:dp + DH, pr, n0:n0 + NS],
                    start=True,
                    stop=True,
                )
            st_tiles[(bi, mt)] = st

        def stage_norm(bi):
            # both heads' sums into one tile, one reciprocal
            po = po_cur[bi]
            sc = rcp.tile([P, NS], F32, tag="sc")
            nc.scalar.copy(sc[0:DH, :], po[0][DH:P, :])
            nc.vector.tensor_copy(sc[DH:P, :], po[1][DH:P, :])
            rc = rcp.tile([P, NS], F32, tag="rc")
            nc.vector.reciprocal_approx_fast(rc[:], sc[:])
            norm_state[bi] = rc

        def norm_tt(bi, i):
            nch, pr = blocks[bi]
            n0 = nch * NS
            rc = norm_state[bi]
            po = po_cur[bi]
            nc.vector.tensor_tensor(
                Ocat[i * DH:(i + 1) * DH, pr, n0:n0 + NS],
                po[i][0:DH, :],
                rc[i * DH:(i + 1) * DH, :],
                mybir.AluOpType.mult,
            )
            if i == 1:
                del po_cur[bi], norm_state[bi]

        # prologue
        emit_S(0, 0)

        for bi, (nch, pr) in enumerate(blocks):
            po = [
                ps_o.tile([P, NS], F32, tag="po", name=f"po{bi}_{i}")
                for i in range(2)
            ]
            po_cur[bi] = po
            lag = 1 if bi + 1 == NB else 2
            for mt in range(MT):
                emit_exp(bi, mt)

                if mt + 1 < MT:
                    emit_S(bi, mt + 1)
                elif bi + 1 < NB:
                    emit_S(bi + 1, 0)
                if bi == 0:
                    if mt % 2 == 0:
                        emit_v(mt // 2)
                    if mt in (3, 5, 7, 9):        # K for head-pair 1
                        emit_k(1, (mt - 3) // 2)
                    elif mt == 11:
                        emit_q(1, 0)
                else:
                    if mt == 0 and bi + 2 < NB:   # Q for the block after next
                        emit_q(blocks[bi + 2][1], blocks[bi + 2][0])
                    if mt == 5 and bi % 2 == 1 and bi >= 3:
                        emit_y(bi // 2 - 1)       # Y: chunk finished 2 blocks ago
                    # previous block's normalize, spread mid-block
                    if mt == 6:
                        stage_norm(bi - 1)
                    elif mt == 8:
                        norm_tt(bi - 1, 0)
                    elif mt == 10:
                        norm_tt(bi - 1, 1)
                if mt >= lag:
                    emit_AV(bi, mt - lag)

            emit_AV(bi, MT - 2)
            emit_AV(bi, MT - 1)

        # tail: last block's normalize + output chunk
        stage_norm(NB - 1)
        norm_tt(NB - 1, 0)
        norm_tt(NB - 1, 1)
        emit_y(N // NS - 1)


def _install_ntff_hook():
    """Best-effort NTFF profiling under axon: provide the antenv.axon_hooks
    shim the boot code looks for, and avoid the artifact upload."""
    try:
        import sys
        import types

        import concourse.bass_utils as bu

        bu.upload_artifacts = lambda d: d  # no S3 in this sandbox
        try:
            from antenv.axon_hooks import get_axon_ntff_profile_hook  # noqa: F401
            return  # already present
        except ImportError:
            pass
        import antenv
        from trn_agent_boot.trn_boot import _ntff_profile_via_ctypes

        mod = types.ModuleType("antenv.axon_hooks")
        _state = {"hook": _ntff_profile_via_ctypes("/opt/axon/libaxon_pjrt.so")}
        mod.set_axon_ntff_profile_hook = lambda h: _state.__setitem__("hook", h)
        mod.get_axon_ntff_profile_hook = lambda: _state["hook"]
        sys.modules["antenv.axon_hooks"] = mod
        antenv.axon_hooks = mod
    except Exception as e:  # pragma: no cover
        print(f"ntff hook install failed ({e}); running without trace")


def _swizzle_dn(a):
    """[D-like, n] -> [128, D/128, n] contiguous (partition-major)."""
    d, n = a.shape
    return np.ascontiguousarray(
        a.reshape(d // P, P, n).transpose(1, 0, 2)).astype(_NP16)


def kernel(x, context, Wq, Wk, Wv, Wo, bo):
    x = np.asarray(x, dtype=np.float32)
    context = np.asarray(context, dtype=np.float32)
    Wq = np.asarray(Wq, dtype=np.float32)
    Wk = np.asarray(Wk, dtype=np.float32)
    Wv = np.asarray(Wv, dtype=np.float32)
    Wo = np.asarray(Wo, dtype=np.float32)
    bo = np.asarray(bo, dtype=np.float32)
    B = x.shape[0]

    in_maps = []
    for c in range(8):
        b, hg = c // 2, c % 2
        sl = slice(hg * DS, (hg + 1) * DS)
        xT = _swizzle_dn(x[b].T)        # [128, 4, 2048]
        cT = _swizzle_dn(context[b].T)
        m = {
            "wq": _swizzle_dn(Wq[:, sl]),
            "wk": _swizzle_dn(Wk[:, sl]),
            "wv": _swizzle_dn(Wv[:, sl]),
            "wo": _swizzle_dn(Wo[sl, :]),
        }
        for i in range(4):
            m[f"xq{i}"] = np.ascontiguousarray(xT[:, :, i * NS:(i + 1) * NS])
            m[f"cq{i}"] = np.ascontiguousarray(cT[:, :, i * NS:(i + 1) * NS])
        in_maps.append(m)

    nc = _build_nc()
    trace = bool(int(os.environ.get("BASS_KERNEL_TRACE", "0")))
    if trace:
        _install_ntff_hook()
    res = run_bass_kernel_spmd(nc, in_maps, list(range(8)), trace=trace)
    if trace and res.exec_time_ns is not None:
        print(f"HW exec time: {res.exec_time_ns} ns")

    out = np.empty((B, N, D), dtype=np.float32)
    for b in range(B):
        # yT: [nch, p, dt, ns] -> y[d, n] with d = dt*128+p, n = nch*512+ns
        yt = (res.results[2 * b]["yT"].astype(np.float32)
              + res.results[2 * b + 1]["yT"].astype(np.float32))
        y = yt.transpose(2, 1, 0, 3).reshape(D, N)
        out[b] = y.T + bo[None, :]
    return out
